# revision 57
# baseline (speedup 1.0000x reference)
"""Single-head attention (no causal mask) on 8 Trainium2 NeuronCores.

Problem: inputs [32, 2048, 64], Wq/Wk/Wv [64, 64] (nn.Linear style, out = x @ W.T).
  q = x @ Wq^T ; k = x @ Wk^T ; v = x @ Wv^T
  out = softmax(q @ k^T / 8) @ v          # no causal mask in the reference

Sharding: data-parallel over batch — 4 batch images per core, weights replicated.

Per-core design (v6):
  - Host pre-transposes x to xT [64, 2048] bf16; weights host-transposed to
    bf16, 1/8 folded into Wq, Wq/Wk duplicated column-wise ([W|W], 64x128) so
    the projection materializes qT/kT twice (partitions 0-63 / 64-127). Even
    k-chunks' score matmuls read the low half (PE row-tile T0), odd chunks the
    high half (T8); the two 64-row tiles stream concurrently.
  - The PE clock gate (HAM) is bistable: once the PE streams without gaps it
    runs at 2.4 GHz, with gaps it sticks at 1.2 GHz. A K=128 warmup burst
    flips it warm at t=0; each batch is processed in two q-half passes so the
    U^T accumulator needs only 2 PSUM banks, freeing 6 banks for a
    3-chunk-deep score pipeline that keeps PE dependencies pre-satisfied.
  - Flat step pipeline over (batch, q-half, chunk): step s emits scores+exp
    for chunk s and AV for chunk s-LAG. Projections for batch b+1 are spread
    into fixed slots; U^T is evacuated in 512-wide quarters inline after the
    final AV matmul of each region (ScalarE/VectorE alternating).
  - exp split between ScalarE (table exp) and VectorE (bf16 Schraudolph:
    bitcast(int16(x*184.665 + 16250.4)); measured end-to-end error ~8e-3
    absmax vs the 2e-2 budget).
  - U^T [65, S] accumulated with lhsT = [v | 1] (row 64 = softmax
    denominator); divide + [h,s]->[s,h] transpose on host.
"""

import math
from contextlib import ExitStack

import numpy as np

import concourse.bass as bass
import concourse.mybir as mybir
import concourse.tile as tile
from concourse import bacc
from concourse.bass import ds, ts
from concourse.bass_utils import run_bass_kernel_spmd

F32 = mybir.dt.float32
BF16 = mybir.dt.bfloat16
I16 = mybir.dt.int16
EXP = mybir.ActivationFunctionType.Exp
MULT = mybir.AluOpType.mult
ADD = mybir.AluOpType.add

B, S, E, H = 32, 2048, 64, 64
NCORES = 8
BC = B // NCORES  # batches per core
NCH = S // 128  # k-chunks per batch
QH = 1024  # q-half width (exp instruction width)
NHALF = S // QH

# Schraudolph bf16 exp: bitcast(int16(x*SCHR_A + SCHR_B)) ~= exp(x)
SCHR_C = 5.6
SCHR_A = 128.0 / math.log(2.0)
SCHR_B = 127.0 * 128.0 - SCHR_C

# exp engine: even k-chunks on ScalarE (table exp), odd on VectorE
# (bf16 Schraudolph) — pairwise-parallel drain, ~8e-3 measured error.
LAG = 2  # AV trails scores by this many chunk-PAIR steps
WARMUP_MMS = 18  # must span one full free-running 3.41us HAM window cold


def build_nc():
    nc = bacc.Bacc("TRN2", target_bir_lowering=False, debug=False)

    xt_d = nc.dram_tensor("xt", [BC, E, S], BF16, kind="ExternalInput").ap()
    wq_d = nc.dram_tensor("wq", [E, 2 * H], BF16, kind="ExternalInput").ap()
    wk_d = nc.dram_tensor("wk", [E, 2 * H], BF16, kind="ExternalInput").ap()
    wv_d = nc.dram_tensor("wv", [E, H], BF16, kind="ExternalInput").ap()
    out_d = nc.dram_tensor("out", [BC, H + 1, S], F32, kind="ExternalOutput").ap()

    ctx = ExitStack()
    with tile.TileContext(nc) as tc:
        with ctx:
            const = ctx.enter_context(tc.tile_pool(name="const", bufs=1))
            xt_pool = ctx.enter_context(tc.tile_pool(name="xt", bufs=3))
            qk_pool = ctx.enter_context(tc.tile_pool(name="qk", bufs=8))
            va_pool = ctx.enter_context(tc.tile_pool(name="va", bufs=2))
            ex_pool = ctx.enter_context(tc.tile_pool(name="ex", bufs=12))
            ut_sb_pool = ctx.enter_context(tc.tile_pool(name="utsb", bufs=4))
            ps_a = ctx.enter_context(tc.tile_pool(name="ps_a", bufs=3, space="PSUM"))
            ps_u = ctx.enter_context(tc.tile_pool(name="ps_u", bufs=1, space="PSUM"))

            ones = const.tile([128, NCH], F32, tag="ones")
            nc.gpsimd.memset(ones[:], 1.0)
            # scratch operands for the warmup burst (results discarded)
            scr_w = const.tile([128, 128], BF16, tag="scr_w")
            scr_x = const.tile([128, 512], BF16, tag="scr_x")
            nc.gpsimd.memset(scr_w[:], 0.0)
            nc.gpsimd.memset(scr_x[:], 0.0)
            # w_all rows 0-63: [Wq|Wq] (q-proj on tile T0); rows 64-127:
            # [Wk|Wk] (k-proj on T8, overlapping q-proj in the array)
            w_all = const.tile([128, 2 * H], BF16, tag="w_all")
            wv_s = const.tile([E, H], BF16, tag="wv")
            nc.sync.dma_start(w_all[:][ds(0, 64), :], wq_d)
            nc.sync.dma_start(w_all[:][ds(64, 64), :], wk_d)
            nc.sync.dma_start(wv_s[:], wv_d)

            # HAM warmup: dependency-free back-to-back full-array matmuls
            warm = ps_a.tile([128, QH], F32, tag="ps")
            for _ in range(WARMUP_MMS):
                nc.tensor.matmul(
                    warm[:, 0:512], scr_w[:], scr_x[:], start=True, stop=True
                )

            def load_xt(b):
                """xT duplicated across partition halves (T0 and T8 reads)."""
                xt_t = xt_pool.tile([128, S], BF16, tag="xt")
                nc.sync.dma_start(xt_t[:][ds(0, 64), :], xt_d[b])
                nc.sync.dma_start(xt_t[:][ds(64, 64), :], xt_d[b])
                return xt_t

            def proj_qk_half(xt_t, h2):
                """(q, k) [128, QH] bf16 for one q-half; each duplicated on
                partitions 0-63 and 64-127. q matmuls run on row-tile T0, k
                on T8, concurrently."""
                ppq = ps_a.tile([128, QH], F32, tag="ps")
                ppk = ps_a.tile([128, QH], F32, tag="ps")
                for j in range(QH // 512):
                    nc.tensor.matmul(
                        ppq[:, ts(j, 512)],
                        w_all[:][ds(0, 64), :],
                        xt_t[:][ds(0, 64), ds(h2 * QH + j * 512, 512)],
                        start=True,
                        stop=True,
                    )
                    nc.tensor.matmul(
                        ppk[:, ts(j, 512)],
                        w_all[:][ds(64, 64), :],
                        xt_t[:][ds(64, 64), ds(h2 * QH + j * 512, 512)],
                        start=True,
                        stop=True,
                    )
                q_t = qk_pool.tile([128, QH], BF16, tag="qk")
                k_t = qk_pool.tile([128, QH], BF16, tag="qk")
                # ScalarE owns these: VectorE's Schraudolph exps are the
                # slower fixed load and must not be displaced by copies
                nc.scalar.copy(q_t[:], ppq[:])
                nc.scalar.copy(k_t[:], ppk[:])
                return q_t, k_t

            def proj_qk(xt_t):
                q_lo, k_lo = proj_qk_half(xt_t, 0)
                q_hi, k_hi = proj_qk_half(xt_t, 1)
                return q_lo, q_hi, k_lo, k_hi

            def proj_v(xt_t):
                """va [128, NCH*65] bf16: per chunk, 64 v-cols + ones col."""
                va = va_pool.tile([128, NCH * 65], BF16, tag="va")
                va_v = va[:].rearrange("p (c w) -> p c w", w=65)
                nc.vector.tensor_copy(
                    va_v[:, :, 64:65],
                    ones[:].rearrange("p (c w) -> p c w", w=1),
                )
                vp = ps_a.tile([128, QH], F32, tag="ps")
                for c in range(NCH):
                    nc.tensor.matmul(
                        vp[:, ts(c, 64)],
                        xt_t[:][ds(0, 64), ts(c, 128)],
                        wv_s[:],
                        start=True,
                        stop=True,
                    )
                src = vp[:].rearrange("p (c w) -> p c w", w=64)
                nc.vector.tensor_copy(va_v[:, :, 0:64], src[:])
                return va

            def scores_pair(qT, kT, t):
                """exp(scores^T) for chunk pair (2t, 2t+1) in one q-half.
                The even chunk streams on row-tile T0, the odd on T8; the
                matmuls are interleaved so both tiles run concurrently.
                Returns (ex_even, ex_odd) [128, QH] bf16."""
                c0 = (2 * t) % 8  # offset within the kT half-tile
                sct0 = ps_a.tile([128, QH], F32, tag="ps")
                sct1 = ps_a.tile([128, QH], F32, tag="ps")
                for j in range(QH // 512):
                    nc.tensor.matmul(
                        sct0[:, ts(j, 512)],
                        kT[:][ds(0, 64), ts(c0, 128)],
                        qT[:][ds(0, 64), ts(j, 512)],
                        start=True,
                        stop=True,
                    )
                    nc.tensor.matmul(
                        sct1[:, ts(j, 512)],
                        kT[:][ds(64, 64), ts(c0 + 1, 128)],
                        qT[:][ds(64, 64), ts(j, 512)],
                        start=True,
                        stop=True,
                    )
                ex0 = ex_pool.tile([128, QH], BF16, tag="ex")
                ex1 = ex_pool.tile([128, QH], BF16, tag="ex")
                nc.scalar.activation(ex0[:], sct0[:], EXP)
                nc.vector.tensor_scalar(
                    ex1[:].bitcast(I16), sct1[:], SCHR_A, SCHR_B, MULT, ADD
                )
                return ex0, ex1

            def av_chunk(ut_ps, ut_sb, b, half, va, ex, c):
                """U^T half += va.T @ ex; inline quarter-evac on last chunk."""
                va_v = va[:].rearrange("p (c w) -> p c w", w=65)
                for j in range(QH // 512):
                    nc.tensor.matmul(
                        ut_ps[0 : H + 1, ts(j, 512)],
                        va_v[:, c, :],
                        ex[:, ts(j, 512)],
                        start=(c == 0),
                        stop=(c == NCH - 1),
                    )
                if c == NCH - 1:
                    nc.scalar.copy(
                        ut_sb[:, ds(half * QH, 512)], ut_ps[0 : H + 1, 0:512]
                    )
                    nc.scalar.copy(
                        ut_sb[:, ds(half * QH + 512, 512)],
                        ut_ps[0 : H + 1, 512:QH],
                    )
                    nc.sync.dma_start(
                        out_d[b][:, ds(half * QH, QH)],
                        ut_sb[:, ds(half * QH, QH)],
                    )

            # prologue: batch 0 projections, xt prefetch for 0 and 1
            xts = {0: load_xt(0), 1: load_xt(1)}
            qks = {0: proj_qk(xts[0])}
            vas = {0: proj_v(xts[0])}
            # bridge burst in the (still idle) U^T accumulator bank: keeps the
            # PE streaming through the projection-evacuation wait so the clock
            # gate stays warm into batch 0; results are overwritten by the
            # first start=True AV matmul
            bridge = ps_u.tile([H + 1, QH], F32, tag="utp")
            for _ in range(12):
                nc.tensor.matmul(
                    bridge[0 : H + 1, 0:512],
                    scr_w[:][:, 0 : H + 1],
                    scr_x[:],
                    start=True,
                    stop=True,
                )


            NPAIR = NCH // 2  # chunk pairs per half
            NSTEP = NHALF * NPAIR  # pair-steps per batch
            exs_all = {}
            ut_cur = {}
            ut_sbs = {}
            flushed = set()
            # the epilogue collapses the AV lag (exps are ready by then), so
            # the loop runs one trailing step fewer
            for s in range(BC * NSTEP + LAG - 1):
                to_flush = [s - LAG]
                if s >= BC * NSTEP - 1:
                    to_flush.append(s - LAG + 1)
                for av in to_flush:
                    if not (0 <= av < BC * NSTEP) or av in flushed:
                        continue
                    flushed.add(av)
                    ba, ra = divmod(av, NSTEP)
                    ha, ta = divmod(ra, NPAIR)
                    if ta == 0:
                        ut_ps = ps_u.tile([H + 1, QH], F32, tag="utp")
                        ut_cur[(ba, ha)] = ut_ps
                        if ha == 0:
                            ut_sb = ut_sb_pool.tile([H + 1, S], F32, tag="ut")
                            ut_sbs[ba] = ut_sb
                    for ca in (2 * ta, 2 * ta + 1):
                        av_chunk(
                            ut_cur[(ba, ha)],
                            ut_sbs[ba],
                            ba,
                            ha,
                            vas[ba],
                            exs_all.pop((ba, ha, ca)),
                            ca,
                        )
                    if ta == NPAIR - 1:
                        ut_cur.pop((ba, ha))
                        if ha == NHALF - 1:
                            ut_sbs.pop(ba)
                if s < BC * NSTEP:
                    b, r = divmod(s, NSTEP)
                    half, t = divmod(r, NPAIR)
                    qT = qks[b][half]  # q_lo / q_hi
                    kT = qks[b][2 + t // 4]  # k_lo / k_hi
                    ex0, ex1 = scores_pair(qT, kT, t)
                    exs_all[(b, half, 2 * t)] = ex0
                    exs_all[(b, half, 2 * t + 1)] = ex1
                    if b + 1 < BC and half == 0:
                        if t == 2:
                            qkh = proj_qk_half(xts[b + 1], 0)
                        elif t == 4:
                            qkh2 = proj_qk_half(xts[b + 1], 1)
                            qks[b + 1] = (qkh[0], qkh2[0], qkh[1], qkh2[1])
                    if b + 1 < BC and half == 1:
                        if t == 2:
                            vas[b + 1] = proj_v(xts[b + 1])
                        if t == 4 and b + 2 < BC:
                            xts[b + 2] = load_xt(b + 2)
                        if t == NPAIR - 1:
                            xts.pop(b, None)

    nc.compile()
    return nc


_NC = None


def _get_nc():
    global _NC
    if _NC is None:
        _NC = build_nc()
    return _NC


def _in_maps(inputs, Wq, Wk, Wv):
    import ml_dtypes

    bf = ml_dtypes.bfloat16
    xt = np.ascontiguousarray(np.transpose(inputs, (0, 2, 1)).astype(bf))
    wq1 = Wq.T.astype(np.float32) / np.float32(np.sqrt(H))
    wq = np.ascontiguousarray(np.concatenate([wq1, wq1], axis=1).astype(bf))
    wk1 = Wk.T.astype(np.float32)
    wk = np.ascontiguousarray(np.concatenate([wk1, wk1], axis=1).astype(bf))
    wv = np.ascontiguousarray(Wv.T.astype(bf))
    return [
        {"xt": xt[c * BC : (c + 1) * BC], "wq": wq, "wk": wk, "wv": wv}
        for c in range(NCORES)
    ]


def run(inputs, Wq, Wk, Wv, **spmd_kwargs):
    nc = _get_nc()
    res = run_bass_kernel_spmd(
        nc, _in_maps(inputs, Wq, Wk, Wv), core_ids=list(range(NCORES)), **spmd_kwargs
    )
    # Each core returns U^T [BC, 65, S]; row 64 is the softmax denominator.
    outs = []
    for r in res.results:
        ut = r["out"]
        outs.append(
            np.transpose(ut[:, :H, :] / ut[:, H : H + 1, :], (0, 2, 1))
        )
    return np.ascontiguousarray(np.concatenate(outs, 0), dtype=np.float32), res


def kernel(inputs, Wq, Wk, Wv):
    out, _ = run(inputs, Wq, Wk, Wv)
    return out
